# revision 1
# baseline (speedup 1.0000x reference)
"""Energy Transformer descent kernel for 8 Trainium2 NeuronCores.

Problem: 12 steps of gradient descent on
  E(x) = -(1/beta) sum logsumexp(beta q k^T) - 0.5 sum relu(g xi^T)^2,
  g = LayerNorm(x; gamma, delta), q = g Wq_h, k = g Wk_h.

Sharding: data-parallel over batch B=4 -> core pairs (2b, 2b+1); within a
pair, core j takes attention heads j*6..j*6+5 and Hopfield memories
xi[j*1536:(j+1)*1536].  Both energy terms contribute additively to dE/dx
and LayerNorm-backward is linear in the upstream gradient, so each core
computes a partial dx and a pairwise AllReduce produces the full step.

Host-side preprocessing folds gamma and the attention scale into the
weights (delta must be zero, which the problem guarantees):
  Wq' = sqrt(beta) diag(gamma) Wq      (forward projections)
  WqT' = (1/sqrt(beta)) (diag(gamma) Wq)^T   (gradient projections)
  xi' = xi diag(gamma)
so the kernel never touches gamma/delta and computes true gradients.

Matmul datapath runs in bf16 (weights quantized host-side; fp32 PSUM
accumulation); the dg accumulation, LayerNorm math and the dx exchange
keep fp32/f32r precision where it matters.  dg is accumulated transposed
([d-chunk, n]) so each accumulation chain owns a full PSUM bank.
"""

import numpy as np

import concourse.bass as bass
import concourse.tile as tile
from concourse import bacc, mybir

STEPS = 12
ALPHA = 0.125
EPS = 1e-5
B, N, D, H, HD, M = 4, 512, 768, 12, 64, 3072
P = 128
NT = N // P  # 4 row chunks
DT = D // P  # 6 embed chunks
HL = H // 2  # heads per core
EW = HL * HD  # 384 local head width
ET = EW // P  # 3 stacked head-pair chunks
ML = M // 2  # memories per core
MT = ML // P  # 12 memory chunks
F32 = mybir.dt.float32
F32R = mybir.dt.float32r
BF16 = mybir.dt.bfloat16
AF = mybir.ActivationFunctionType
OP = mybir.AluOpType

REPLICA_GROUPS = [[0, 1], [2, 3], [4, 5], [6, 7]]


def f_(ap):
    return ap.bitcast(F32)




def build_kernel(steps=STEPS, with_ar=True, debug_phase=99, debug_dump=False):
    nc = bacc.Bacc("TRN2", target_bir_lowering=False, debug=False, num_devices=8)

    x_in = nc.declare_dram_parameter("x", [N, D], F32, isOutput=False)
    wq_d = nc.declare_dram_parameter("wq", [D, EW], BF16, isOutput=False)
    wk_d = nc.declare_dram_parameter("wk", [D, EW], BF16, isOutput=False)
    wqt_d = nc.declare_dram_parameter("wqt", [EW, D], BF16, isOutput=False)
    wkt_d = nc.declare_dram_parameter("wkt", [EW, D], BF16, isOutput=False)
    xi_d = nc.declare_dram_parameter("xi", [ML, D], BF16, isOutput=False)
    xit_d = nc.declare_dram_parameter("xit", [D, ML], BF16, isOutput=False)
    x_out = nc.declare_dram_parameter("x_out", [N, D], F32, isOutput=True)
    dbg = {}
    if debug_dump:
        for nm, shp in (("xhat", [N, D]), ("gT", [D, N]), ("q", [N, EW]), ("kT", [EW, N]),
                        ("P0", [N, N]), ("dqT", [EW, N]), ("dg", [N, D]), ("dx", [N, D])):
            dbg[nm] = nc.declare_dram_parameter("o_" + nm, shp, F32, isOutput=True)

    with tile.TileContext(nc) as tc:
        import contextlib

        with contextlib.ExitStack() as ctx:
            consts = ctx.enter_context(tc.tile_pool(name="consts", bufs=1))
            work = ctx.enter_context(tc.tile_pool(name="work", bufs=1))
            attp = ctx.enter_context(tc.tile_pool(name="attp", bufs=2))
            stats = ctx.enter_context(tc.tile_pool(name="stats", bufs=4))
            stream = ctx.enter_context(tc.tile_pool(name="stream", bufs=3))
            rtp = ctx.enter_context(tc.tile_pool(name="rtp", bufs=3))
            scr = ctx.enter_context(tc.tile_pool(name="scr", bufs=2))
            ps = ctx.enter_context(tc.tile_pool(name="ps", bufs=2, space="PSUM"))
            drp = ctx.enter_context(tc.tile_pool(name="drp", bufs=2, space="DRAM"))

            # ---- resident tensors ----
            wq_sb = consts.tile([P, DT, EW], BF16)
            nc.sync.dma_start(out=wq_sb[:], in_=wq_d.rearrange("(dt p) e -> p dt e", p=P))
            wk_sb = consts.tile([P, DT, EW], BF16)
            nc.sync.dma_start(out=wk_sb[:], in_=wk_d.rearrange("(dt p) e -> p dt e", p=P))
            wqt_sb = consts.tile([P, ET, D], BF16)
            nc.sync.dma_start(out=wqt_sb[:], in_=wqt_d.rearrange("(et p) d -> p et d", p=P))
            wkt_sb = consts.tile([P, ET, D], BF16)
            nc.sync.dma_start(out=wkt_sb[:], in_=wkt_d.rearrange("(et p) d -> p et d", p=P))
            x_sb = consts.tile([P, NT, D], F32)
            nc.sync.dma_start(out=x_sb[:], in_=x_in.rearrange("(nt p) d -> p nt d", p=P))

            from concourse.masks import make_identity

            ident_f = consts.tile([P, P], F32)
            make_identity(nc, ident_f[:])
            ident = consts.tile([P, P], F32R)
            nc.vector.tensor_copy(out=ident[:], in_=ident_f[:])
            ident_b = consts.tile([P, P], BF16)
            nc.vector.tensor_copy(out=ident_b[:], in_=ident_f[:])
            eps_t = consts.tile([P, 1], F32)
            nc.vector.memset(eps_t[:], EPS)

            for step in range(steps):
                # ======== LayerNorm forward ========
                xhat = work.tile([P, NT, D], F32R, tag="xhat")
                rstd = stats.tile([P, NT], F32, tag="rstd")
                for nt in range(NT):
                    xt = x_sb[:, nt, :]
                    st = stats.tile([P, 3, 6], F32, tag="bnst")
                    xg = xt.rearrange("p (g s) -> p g s", s=256)
                    for gs in range(3):
                        nc.vector.bn_stats(out=st[:, gs, :], in_=xg[:, gs, :])
                    mv = stats.tile([P, 2], F32, tag="mv")
                    nc.vector.bn_aggr(out=mv[:], in_=st[:])
                    rr = rstd[:, nt : nt + 1]
                    nc.scalar.activation(out=rr, in_=mv[:, 1:2], func=AF.Sqrt, bias=eps_t[:], scale=1.0)
                    nc.vector.reciprocal(out=rr, in_=rr)
                    nmu = stats.tile([P, 1], F32, tag="nmu")
                    nc.vector.scalar_tensor_tensor(
                        out=nmu[:], in0=mv[:, 0:1], scalar=-1.0, in1=rr, op0=OP.mult, op1=OP.mult,
                    )
                    nc.scalar.activation(
                        out=xhat[:, nt, :], in_=xt, func=AF.Identity, scale=rr, bias=nmu[:],
                    )

                if debug_phase < 2:
                    continue
                # gT = xhat^T  [d-part, n-free]
                psw_ctx = tc.tile_pool(name="psw", bufs=6, space="PSUM")
                psw = psw_ctx.__enter__()
                gT = work.tile([P, DT, N], BF16, tag="gT")
                for dt in range(DT):
                    pt = psw.tile([P, 512], F32R, tag="psw")
                    for nt in range(NT):
                        nc.tensor.transpose(pt[:, nt * P : (nt + 1) * P], xhat[:, nt, dt * P : (dt + 1) * P], ident[:])
                    nc.vector.tensor_copy(out=gT[:, dt, :], in_=pt[:])

                if debug_dump and step == 0:
                    nc.sync.dma_start(out=dbg["xhat"].rearrange("(nt p) d -> p nt d", p=P), in_=f_(xhat[:]))
                    nc.sync.dma_start(out=dbg["gT"].rearrange("(dt p) n -> p dt n", p=P), in_=f_(gT[:]))
                if debug_phase < 3:
                    continue
                # ======== projections ========
                q = work.tile([P, NT, EW], BF16, tag="q")
                k = work.tile([P, NT, EW], BF16, tag="k")
                for nt in range(NT):
                    ppq = psw.tile([P, 512], F32, tag="psw")
                    ppk = psw.tile([P, 512], F32, tag="psw")
                    for dt in range(DT):
                        lh = gT[:, dt, nt * P : (nt + 1) * P]
                        nc.tensor.matmul(ppq[:, :EW], lh, wq_sb[:, dt, :], start=(dt == 0), stop=(dt == DT - 1))
                        nc.tensor.matmul(ppk[:, :EW], lh, wk_sb[:, dt, :], start=(dt == 0), stop=(dt == DT - 1))
                    nc.vector.tensor_copy(out=q[:, nt, :], in_=ppq[:, :EW])
                    nc.vector.tensor_copy(out=k[:, nt, :], in_=ppk[:, :EW])
                qT = work.tile([P, ET, N], BF16, tag="qT")
                kT = work.tile([P, ET, N], BF16, tag="kT")
                for dst, srct in ((qT, q), (kT, k)):
                    for et in range(ET):
                        pp = psw.tile([P, 512], BF16, tag="psw")
                        for nt in range(NT):
                            nc.tensor.transpose(
                                pp[:, nt * P : (nt + 1) * P],
                                srct[:, nt, et * P : (et + 1) * P], ident_b[:],
                            )
                        nc.vector.tensor_copy(out=dst[:, et, :], in_=pp[:])

                if debug_dump and step == 0:
                    nc.sync.dma_start(out=dbg["q"].rearrange("(nt p) e -> p nt e", p=P), in_=f_(q[:]))
                    nc.sync.dma_start(out=dbg["kT"].rearrange("(et p) n -> p et n", p=P), in_=f_(kT[:]))
                if debug_phase < 4:
                    continue
                # ======== attention heads ========
                dqTst = work.tile([P, ET, N], BF16, tag="dqTst")
                dkTst = work.tile([P, ET, N], BF16, tag="dkTst")
                for h in range(HL):
                    et, eo = h // 2, (h % 2) * HD
                    Pn = attp.tile([P, NT, N], BF16, tag="Pn")
                    PTn = attp.tile([P, NT, N], BF16, tag="PTn")
                    for nt in range(NT):
                        sc = psw.tile([P, 512], F32, tag="psw")
                        nc.tensor.matmul(
                            sc[:], qT[eo : eo + HD, et, nt * P : (nt + 1) * P],
                            kT[eo : eo + HD, et, :], start=True, stop=True,
                        )
                        sm = stats.tile([P, 1], F32, tag="sm")
                        nc.scalar.activation(
                            out=Pn[:, nt, :], in_=sc[:], func=AF.Exp, bias=0.0, scale=1.0,
                            accum_out=sm[:],
                        )
                        nc.vector.reciprocal(out=sm[:], in_=sm[:])
                        nc.vector.tensor_scalar_mul(out=Pn[:, nt, :], in0=Pn[:, nt, :], scalar1=sm[:])
                    if debug_dump and step == 0 and h == 0:
                        nc.sync.dma_start(out=dbg["P0"].rearrange("(nt p) m -> p nt m", p=P), in_=f_(Pn[:]))
                    # PT via PE transposes (4 transposes share one psum tile)
                    for mt in range(NT):
                        pt = psw.tile([P, 512], BF16, tag="psw")
                        for nt in range(NT):
                            nc.tensor.transpose(pt[:, nt * P : (nt + 1) * P], Pn[:, nt, mt * P : (mt + 1) * P], ident_b[:])
                        nc.vector.tensor_copy(out=PTn[:, mt, :], in_=pt[:])
                    # dqT_h = sum_mt k_h[mt]^T-as-lhsT @ PT[mt]
                    pp = psw.tile([P, 512], F32, tag="psw")
                    for mt in range(NT):
                        nc.tensor.matmul(
                            pp[:HD, :], k[:, mt, h * HD : (h + 1) * HD], PTn[:, mt, :],
                            start=(mt == 0), stop=(mt == NT - 1),
                        )
                    nc.vector.tensor_copy(out=dqTst[eo : eo + HD, et, :], in_=pp[:HD, :])
                    # dkT_h = sum_nt q_h[nt]-as-lhsT @ P[nt]
                    pp2 = psw.tile([P, 512], F32, tag="psw")
                    for nt in range(NT):
                        nc.tensor.matmul(
                            pp2[:HD, :], q[:, nt, h * HD : (h + 1) * HD], Pn[:, nt, :],
                            start=(nt == 0), stop=(nt == NT - 1),
                        )
                    nc.vector.tensor_copy(out=dkTst[eo : eo + HD, et, :], in_=pp2[:HD, :])

                if debug_dump and step == 0:
                    nc.sync.dma_start(out=dbg["dqT"].rearrange("(et p) n -> p et n", p=P), in_=f_(dqTst[:]))
                psw_ctx.__exit__(None, None, None)
                if debug_phase < 5:
                    continue
                # ======== dg accumulation in PSUM, transposed [d-chunk, n] ========
                # dgT (= -true dg^T): each d-chunk owns a full PSUM bank so every
                # accumulation chain is bank-exclusive (PSUM has_written clears are
                # bank-wide; two chains must never share a bank).
                psdg_ctx = tc.tile_pool(name="psdg", bufs=1, space="PSUM")
                psdg = psdg_ctx.__enter__()
                dgTb = [psdg.tile([P, N], F32, tag=f"dgT{dt}", name=f"dgT{dt}") for dt in range(DT)]
                for dt in range(DT):
                    first = True
                    for et in range(ET):
                        for d_t, w_t in ((dqTst, wqt_sb), (dkTst, wkt_sb)):
                            nc.tensor.matmul(
                                dgTb[dt][:], w_t[:, et, dt * P : (dt + 1) * P],
                                d_t[:, et, :], start=first, stop=False,
                            )
                            first = False

                # ======== hopfield ========
                for mt in range(MT):
                    xitm = stream.tile([P, DT, P], BF16, tag="xitm")
                    nc.sync.dma_start(
                        out=xitm[:],
                        in_=xit_d[:, mt * P : (mt + 1) * P].rearrange("(dt p) m -> p dt m", p=P),
                    )
                    hp = ps.tile([P, 512], F32, tag="ps")
                    for dt in range(DT):
                        nc.tensor.matmul(
                            hp[:], xitm[:, dt, :], gT[:, dt, :],
                            start=(dt == 0), stop=(dt == DT - 1),
                        )
                    RT = rtp.tile([P, N], BF16, tag="RT")
                    nc.scalar.activation(out=RT[:], in_=hp[:], func=AF.Relu)
                    xim = stream.tile([P, D], BF16, tag="xim")
                    nc.sync.dma_start(out=xim[:], in_=xi_d[mt * P : (mt + 1) * P, :])
                    last = mt == MT - 1
                    for dt in range(DT):
                        nc.tensor.matmul(
                            dgTb[dt][:], xim[:, dt * P : (dt + 1) * P], RT[:],
                            start=False, stop=last,
                        )

                if debug_phase < 7:
                    continue
                # ======== transpose dg back to [n-part, d] ========
                dgTs = work.tile([P, DT, N], F32R, tag="dgTs")
                for dt in range(DT):
                    nc.vector.tensor_copy(out=dgTs[:, dt, :], in_=dgTb[dt][:])
                psdg_ctx.__exit__(None, None, None)
                dx = work.tile([P, NT, D], F32, tag="dx")
                dxb = work.tile([P, NT, D], BF16, tag="dxb")
                m1s = stats.tile([P, 2, NT], F32, tag="m1s")
                for nt in range(NT):
                    pt = ps.tile([P, 512], F32R, tag="ps")
                    for dt in range(4):
                        nc.tensor.transpose(pt[:, dt * P : (dt + 1) * P], dgTs[:, dt, nt * P : (nt + 1) * P], ident[:])
                    nc.vector.scalar_tensor_tensor(
                        out=dx[:, nt, 0:512], in0=f_(pt[:]), scalar=0.0, in1=xhat[:, nt, 0:512].bitcast(F32),
                        op0=OP.bypass, op1=OP.bypass, accum_out=m1s[:, 0, nt : nt + 1],
                    )
                    pt2 = ps.tile([P, 512], F32R, tag="ps")
                    for dt in range(4, DT):
                        nc.tensor.transpose(pt2[:, (dt - 4) * P : (dt - 3) * P], dgTs[:, dt, nt * P : (nt + 1) * P], ident[:])
                    nc.vector.scalar_tensor_tensor(
                        out=dx[:, nt, 512:768], in0=f_(pt2[:, :256]), scalar=0.0, in1=xhat[:, nt, 512:768].bitcast(F32),
                        op0=OP.bypass, op1=OP.bypass, accum_out=m1s[:, 1, nt : nt + 1],
                    )
                if debug_dump and step == 0:
                    nc.sync.dma_start(out=dbg["dg"].rearrange("(nt p) d -> p nt d", p=P), in_=dx[:])

                # ======== LayerNorm backward (in place on dx; dx holds dg) ========
                for nt in range(NT):
                    rr = rstd[:, nt : nt + 1]
                    m1 = stats.tile([P, 1], F32, tag="m1")
                    nc.vector.tensor_tensor(out=m1[:], in0=m1s[:, 0, nt : nt + 1], in1=m1s[:, 1, nt : nt + 1], op=OP.add)
                    prodA = scr.tile([P, D], F32, tag="prodA")
                    u2 = stats.tile([P, 1], F32, tag="u2")
                    nc.vector.scalar_tensor_tensor(
                        out=prodA[:], in0=dx[:, nt, :], scalar=1.0, in1=f_(xhat[:, nt, :]),
                        op0=OP.mult, op1=OP.mult, accum_out=u2[:],
                    )
                    c1 = stats.tile([P, 1], F32, tag="c1")
                    nc.vector.scalar_tensor_tensor(
                        out=c1[:], in0=m1[:], scalar=1.0 / D, in1=rr, op0=OP.mult, op1=OP.mult,
                    )
                    c2 = stats.tile([P, 1], F32, tag="c2")
                    nc.vector.scalar_tensor_tensor(
                        out=c2[:], in0=u2[:], scalar=-1.0 / D, in1=rr, op0=OP.mult, op1=OP.mult,
                    )
                    lnv = scr.tile([P, D], F32, tag="lnv")
                    nc.vector.tensor_scalar(
                        out=lnv[:], in0=dx[:, nt, :], scalar1=rr, scalar2=c1[:],
                        op0=OP.mult, op1=OP.subtract,
                    )
                    nc.vector.scalar_tensor_tensor(
                        out=dxb[:, nt, :], in0=f_(xhat[:, nt, :]), scalar=c2[:], in1=lnv[:],
                        op0=OP.mult, op1=OP.add,
                    )

                if debug_dump and step == 0:
                    nc.sync.dma_start(out=dbg["dx"].rearrange("(nt p) d -> p nt d", p=P), in_=dx[:])
                # ======== pair AllReduce + update ========
                if with_ar:
                    arin = drp.tile([N, D], BF16, tag="arin")
                    arout = drp.tile([N, D], BF16, tag="arout")
                    for nt in range(NT):
                        nc.sync.dma_start(out=arin[nt * P : (nt + 1) * P, :], in_=dxb[:, nt, :])
                    nc.gpsimd.collective_compute(
                        "AllReduce", OP.add, replica_groups=REPLICA_GROUPS,
                        ins=[arin.opt()], outs=[arout.opt()],
                    )
                    nc.sync.dma_start(out=dxb[:], in_=arout.rearrange("(nt p) d -> p nt d", p=P))
                for nt in range(NT):
                    nc.vector.tensor_copy(out=dx[:, nt, :], in_=dxb[:, nt, :])
                upd = dx
                if debug_phase < 12:
                    continue
                for nt in range(NT):
                    nc.vector.scalar_tensor_tensor(
                        out=x_sb[:, nt, :], in0=upd[:, nt, :], scalar=ALPHA, in1=x_sb[:, nt, :],
                        op0=OP.mult, op1=OP.add,
                    )

            for nt in range(NT):
                nc.sync.dma_start(out=x_out[nt * P : (nt + 1) * P, :], in_=x_sb[:, nt, :])

    nc.compile()
    return nc


def _prep_inputs(x, gamma, delta, Wq, Wk, xi):
    """Build the 8 per-core input dicts (host-side sharding + weight folding)."""
    assert np.allclose(delta, 0.0), "kernel requires delta == 0"
    beta_sqrt = np.float32(1.0 / np.sqrt(np.sqrt(np.float32(HD))))
    # sqrt(beta) = (1/sqrt(HD))^(1/2) = HD^(-1/4)
    g = gamma.astype(np.float32)
    in_maps = []
    for c in range(8):
        b, j = c // 2, c % 2
        hs = slice(j * HL, (j + 1) * HL)
        wq_l = (Wq[hs] * g[None, :, None]).transpose(1, 0, 2).reshape(D, EW)
        wk_l = (Wk[hs] * g[None, :, None]).transpose(1, 0, 2).reshape(D, EW)
        wqt_l = (Wq[hs] * g[None, :, None]).transpose(0, 2, 1).reshape(EW, D)
        wkt_l = (Wk[hs] * g[None, :, None]).transpose(0, 2, 1).reshape(EW, D)
        xi_l = xi[j * ML : (j + 1) * ML] * g[None, :]
        import ml_dtypes

        bf = ml_dtypes.bfloat16
        in_maps.append(
            {
                "x": np.ascontiguousarray(x[b]),
                "wq": np.ascontiguousarray(wq_l * beta_sqrt).astype(bf),
                "wk": np.ascontiguousarray(wk_l * beta_sqrt).astype(bf),
                "wqt": np.ascontiguousarray(wqt_l / beta_sqrt).astype(bf),
                "wkt": np.ascontiguousarray(wkt_l / beta_sqrt).astype(bf),
                "xi": np.ascontiguousarray(xi_l).astype(bf),
                "xit": np.ascontiguousarray(xi_l.T).astype(bf),
            }
        )
    return in_maps


_NC_CACHE = {}


def _get_nc(steps=STEPS, with_ar=True):
    key = (steps, with_ar)
    if key not in _NC_CACHE:
        _NC_CACHE[key] = build_kernel(steps, with_ar)
    return _NC_CACHE[key]


def kernel(x, gamma, delta, Wq, Wk, xi):
    from concourse.bass_utils import run_bass_kernel_spmd

    x = np.asarray(x, dtype=np.float32)
    in_maps = _prep_inputs(
        x,
        np.asarray(gamma, np.float32),
        np.asarray(delta, np.float32),
        np.asarray(Wq, np.float32),
        np.asarray(Wk, np.float32),
        np.asarray(xi, np.float32),
    )
    nc = _get_nc()
    res = run_bass_kernel_spmd(nc, in_maps, list(range(8)))
    out = np.stack([res.results[2 * b]["x_out"] for b in range(B)], axis=0)
    return out.astype(np.float32)

